# revision 30
# baseline (speedup 1.0000x reference)
import sys

sys.path.insert(0, "/opt/trn_rl_repo")

import numpy as np
import ml_dtypes

import concourse.mybir as mybir
from concourse import bass, tile
from concourse import tile_sem_assignment as _tsa
from concourse.bass_utils import run_bass_kernel_spmd
from concourse.vector_clock import ScopedClock, VectorClock

_orig_drain_and_barrier = tile.TileContext._drain_and_barrier


def _split_drain_and_barrier(self, tick_clock, wait_clock):
    # The final Drain waits on every active semaphore at once; with 8 HWDGE
    # lanes + SWDGE + 3 engines that exceeds the CTRL instruction's sync
    # wait slots. Emit one 1-wait drain per proc instead (same semantics:
    # SP executes them in order, so all sems reach their targets before the
    # barrier), then replicate the original barrier/cleanup sequence.
    gc = tick_clock.global_clock
    n = _tsa.N_PROCS
    for p in range(n):
        if gc[p] > 0:
            partial = VectorClock([gc[q] if q == p else 0 for q in range(n)])
            d = self.nc.sync.drain()
            wait_clock.add_sem_waits(d.ins, ScopedClock({None: partial}))
    self.nc.all_engine_barrier()
    popped = self.nc._tile_sem_poison_stack.pop()
    assert popped is self._sem_poison
    self.nc.clear_and_free_semaphores(list(self.sems.allocated().values()))
    self.nc.all_engine_barrier()


tile.TileContext._drain_and_barrier = _split_drain_and_barrier

B = 1024        # batch rows of address
N = 65536       # mem rows (sharded)
M = 128         # mem cols
NCORES = 8
NS = N // NCORES          # 8192 rows per core
NCHUNKS = NS // 128       # 64 chunks of 128 mem-rows
MCHUNKS = NS // 256       # 32 mega-chunks of 256 mem-rows (DoubleRow)
BCHUNKS = B // 128        # 8 chunks of 128 batch-rows
NSTAGES = 8               # DMA pipeline stages (8 chunks each)

FP8 = mybir.dt.float8e4
BF16 = mybir.dt.bfloat16
F32 = mybir.dt.float32
DR = mybir.MatmulPerfMode.DoubleRow
ADD = mybir.AluOpType.add
MULT = mybir.AluOpType.mult

_compiled = {}


NG = 16  # DMA groups; each covers 4 mem-chunks (k) = 2 mega-chunks (ch)


def _build_nc():
    nc = bass.Bass(target_bir_lowering=False)

    # a:  [p=b%128, j(n-slice of 1024), ub(u-block), bc, u]  A shard for GEMM1
    a = nc.dram_tensor("a", [128, NSTAGES, 8, BCHUNKS, 128], FP8, kind="ExternalInput")
    # at: [p=n%128 within 256-chunk, ch, sub, b]  A^T shard for GEMM2 (partition=n)
    at = nc.dram_tensor("at", [128, MCHUNKS, 2, B], FP8, kind="ExternalInput")
    # c:  [p=n%128, k, m]  0.5*content shard (partition=n)
    c = nc.dram_tensor("c", [128, NCHUNKS, M], FP8, kind="ExternalInput")
    # ed: [p=b%128, h(hi/lo), bc, 2M]  [-erase | 0.5*add] split as hi+lo fp8
    ed = nc.dram_tensor("ed", [128, 2, BCHUNKS, 2 * M], FP8, kind="ExternalInput")
    # rt: [m, b] partial (read/2)^T fp32
    rt = nc.dram_tensor("rt", [M, B], F32, kind="ExternalOutput")

    with tile.TileContext(nc) as tc:
        with (
            tc.tile_pool(name="abuf", bufs=1) as a_pool,
            tc.tile_pool(name="atbuf", bufs=1) as at_pool,
            tc.tile_pool(name="cbuf", bufs=1) as c_pool,
            tc.tile_pool(name="edbuf", bufs=1) as ed_pool,
            tc.tile_pool(name="tmpbuf", bufs=8) as tmp_pool,
            tc.tile_pool(name="cpbuf", bufs=6) as cp_pool,
            tc.tile_pool(name="rtbuf", bufs=1) as rt_pool,
            tc.tile_pool(name="pw", bufs=6, space="PSUM") as pw_pool,
            tc.tile_pool(name="pr", bufs=1, space="PSUM") as pr_pool,
        ):
            a_t = a_pool.tile([128, NSTAGES, 8, BCHUNKS, 128], FP8)
            at_t = at_pool.tile([128, MCHUNKS, 2, B], FP8)
            c_t = c_pool.tile([128, NCHUNKS, M], FP8)
            ed_t = ed_pool.tile([128, 2, BCHUNKS, 2 * M], FP8)

            # Fine-grained preloads in consumption order so DMA arrival
            # tracks compute need and the scheduler interleaves G1/G2
            # naturally (the wait-dedup chain needs G2 close behind G1).
            # Preload DMAs write each SBUF dest exactly once, so their only
            # wait is the HWDGE lane-credit wait (1 wait, allowed). The rt
            # store goes out over SWDGE (gpsimd) so it lands on a fresh
            # lane and carries only its RAW wait.
            def a_group(g):
                j, ub0 = g // 2, (g % 2) * 4
                nc.sync.dma_start(
                    out=a_t[:, j, ub0 : ub0 + 4], in_=a[:, j, ub0 : ub0 + 4]
                )

            # 'a' leads 'at' by one group so the G1->STT->TADD chain for the
            # final chunks drains while the last at groups are still in
            # flight; the last at group is split per mega-chunk so only
            # G2(31) + copy/store trail the final DMA.
            nc.sync.dma_start(out=ed_t[:], in_=ed[:])
            nc.sync.dma_start(out=c_t[:, 0:32, :], in_=c[:, 0:32, :])
            a_group(0)
            for g in range(NG - 2):
                a_group(g + 1)
                if g == 7:
                    nc.sync.dma_start(out=c_t[:, 32:64, :], in_=c[:, 32:64, :])
                nc.sync.dma_start(
                    out=at_t[:, 2 * g : 2 * g + 2], in_=at[:, 2 * g : 2 * g + 2]
                )
            a_group(NG - 1)
            nc.sync.dma_start(out=at_t[:, 28:30], in_=at[:, 28:30])
            nc.sync.dma_start(out=at_t[:, 30:31], in_=at[:, 30:31])
            nc.sync.dma_start(out=at_t[:, 31:32], in_=at[:, 31:32])

            psum_r = pr_pool.tile([128, B], F32)
            land = tmp_pool.tile([128, 1], F32)
            # Wake the Activation engine early: its first instruction carries
            # a ~1.4us startup cost in the model; pay it off the critical
            # path so the tail copies run at steady-state rate.
            warm = tmp_pool.tile([128, 1], F32)
            nc.scalar.copy(warm[:], ed_t[:, 0, 0, 0:1])

            def emit_g2(ch, cp):
                for jj in range(2):
                    nc.tensor.matmul(
                        psum_r[:, jj * 512 : (jj + 1) * 512],
                        cp[:],
                        at_t[:, ch, :, jj * 512 : (jj + 1) * 512],
                        start=(ch == 0),
                        stop=(ch == MCHUNKS - 1),
                        perf_mode=DR,
                    )

            cp = None
            for k in range(NCHUNKS):
                ch, sub = k // 2, k % 2
                j, ub = k // 8, k % 8
                if k % 32 == 0:
                    # DVE absorbs this c-half's DMA wait so STT(k) keeps
                    # only its PSUM-read wait (dedup on the same lane sem).
                    nc.vector.tensor_copy(land[:], c_t[:, k, 0:1])

                if sub == 0:
                    cp = cp_pool.tile([128, 2, M], FP8)

                psum_w = pw_pool.tile([128, 2 * M], F32)
                for h in range(2):
                    for q in range(4):
                        nc.tensor.matmul(
                            psum_w[:],
                            a_t[:, j, ub, 2 * q : 2 * q + 2, :],
                            ed_t[:, h, 2 * q : 2 * q + 2, :],
                            start=(h == 0 and q == 0),
                            stop=(h == 1 and q == 3),
                            perf_mode=DR,
                        )

                # psum_w = [-We | Wa/2];  C'/2 = (1 - We) * (C/2) + Wa/2
                tmp2 = tmp_pool.tile([128, M], F32)
                nc.vector.scalar_tensor_tensor(
                    tmp2[:], psum_w[:, 0:M], 1.0, c_t[:, k, :], ADD, MULT
                )
                nc.vector.tensor_add(cp[:, sub, :], tmp2[:], psum_w[:, M : 2 * M])

                if sub == 1:
                    # G2 Ldweights (stationary=cp) carries DVE>=tadd(2ch+1),
                    # covering the bank-WAR waits of later G1 start-matmuls
                    # via per-engine wait dedup.
                    emit_g2(ch, cp)

            # Split the tail: psum_r bank jj completes at G2(ch=31, jj), so
            # copy+store each half as soon as its accumulation stops instead
            # of one serial full-width copy followed by one big store.
            rt_t = rt_pool.tile([128, B], F32)
            for jj in range(2):
                nc.scalar.copy(
                    rt_t[:, jj * 512 : (jj + 1) * 512],
                    psum_r[:, jj * 512 : (jj + 1) * 512],
                )
                nc.gpsimd.dma_start(
                    out=rt[:, jj * 512 : (jj + 1) * 512],
                    in_=rt_t[:, jj * 512 : (jj + 1) * 512],
                )

    # The scheduler can hoist a G1 start-Matmult ahead of the G2 Ldweights
    # whose DVE wait would dedup-cover its bank-WAR wait, leaving it with
    # two waits (PE self-wait + DVE) — one over the HW wait-slot limit.
    # The same-engine self-wait is always satisfied by in-order queue
    # completion, so drop it.
    for inst in nc.inst_map.values():
        si = inst.sync_info
        if si and si.on_wait and len(si.on_wait) > 1:
            eng = str(inst.engine).split(".")[-1]
            kept = [w for w in si.on_wait if not w.ant_name.startswith(eng + "_")]
            assert len(kept) == 1
            si.on_wait = kept

    return nc


def _prep_inputs(address, erase, add, content):
    f8 = ml_dtypes.float8_e4m3
    a_f8 = address.astype(f8)                                 # [1024, 65536]
    ed = np.concatenate([-erase, 0.5 * add], axis=1)          # [1024, 256] f32
    ed_hi = ed.astype(f8)
    ed_lo = (ed - ed_hi.astype(np.float32)).astype(f8)
    ed_st = np.stack([ed_hi, ed_lo])                          # [2, 1024, 256]
    ed_r = np.ascontiguousarray(
        ed_st.reshape(2, BCHUNKS, 128, 2 * M).transpose(2, 0, 1, 3)
    )                                                         # [128, 2, 8, 256]
    c_bf = (0.5 * content).astype(f8)                         # [65536, 128]

    in_maps = []
    for ci in range(NCORES):
        a_c = a_f8[:, ci * NS : (ci + 1) * NS]                # [1024, 8192]
        # a_r[p, j, ub, bc, u] = A[bc*128+p, j*1024+ub*128+u]
        a_r = np.ascontiguousarray(
            a_c.reshape(BCHUNKS, 128, NSTAGES, 8, 128).transpose(1, 2, 3, 0, 4)
        )                                                     # [128, 8, 8, 8, 128]
        # at_r[p, ch, s, b] = A[b, ch*256 + s*128 + p]
        at_r = np.ascontiguousarray(
            a_c.T.reshape(MCHUNKS, 2, 128, B).transpose(2, 0, 1, 3)
        )                                                     # [128, 32, 2, 1024]
        c_c = c_bf[ci * NS : (ci + 1) * NS, :]
        c_r = np.ascontiguousarray(
            c_c.reshape(NCHUNKS, 128, M).transpose(1, 0, 2)
        )                                                     # [128, 64, 128]
        in_maps.append({"a": a_r, "at": at_r, "c": c_r, "ed": ed_r})
    return in_maps


def kernel(address, erase, add, content, _trace=False, _result_box=None):
    if "nc" not in _compiled:
        _compiled["nc"] = _build_nc()
    nc = _compiled["nc"]

    in_maps = _prep_inputs(address, erase, add, content)
    res = run_bass_kernel_spmd(
        nc, in_maps, core_ids=list(range(NCORES)), trace=_trace
    )
    if _result_box is not None:
        _result_box.append(res)

    acc = np.zeros((M, B), dtype=np.float32)
    for r in res.results:
        acc += np.asarray(r["rt"], dtype=np.float32)
    return np.ascontiguousarray((2.0 * acc).T)
